# revision 13
# baseline (speedup 1.0000x reference)
"""Trainium2 Bass kernel for nn_DirectionalWedgeBias.

Computes, per (batch b, head h):
    v      = x[b].reshape(T, H, Dh)[:, h, :]          # [T, Dh]
    v_hat  = v / max(||v||_2, eps)  (row-wise)
    S      = A[h] - A[h]^T                            # [Dh, Dh]
    wedge  = (v_hat @ S) @ v_hat^T                    # [T, T]

Full shapes: x [2, 2048, 1024] f32, A [16, 64, 64] f32 -> out [2, 16, 2048, 2048] f32.

Sharding: 32 independent (b, h) pairs split 4-per-core across 8 NeuronCores
(data + head parallel; the tiny skew-symmetric S is replicated/sliced with the
heads). Host pre-slices x into per-core [4, T, Dh] blocks, forms S = A - A^T,
and re-stacks the per-core [4, T, T] results.

Per-core dataflow (Tile framework), redesigned around the v1 cost model:

  - The wedge math runs in bf16 (v_hat, S, SvT all bf16; rel err ~5e-3 vs the
    2e-2 budget): bf16 matmuls and transposes run at 1 cyc/row on the PE, and
    DVE cast-copies/all-bf16 elementwise ops hit the 2x perf modes.
  - Interleaved m-blocks: wedge row-block m uses lhsT = svt16[:, m::16]
    (columns m, m+16, ...), so PSUM partition q of block m holds output row
    t = q*16 + m.  A group's 16 m-blocks staged at [128 part, 16, 512] in
    SBUF then flatten to *sequential* DRAM rows: the store's DRAM-side AP
    balances to [[rows, 2048], [1, 1], [1, 512]], whose free size (and hence
    DMA queue cost) is 512 elems -> ~0.8 us per 4 MiB store instead of the
    ~50 us a [128, ...]-leading AP costs.  The staging tile keeps a 516-elem
    chunk stride so its free dims cannot re-merge during AP balancing.
  - PSUM->SBUF evacuation (the 64 MiB wedge + SvT/vT) is the #2 cost center:
    copies are spread across ACT/DVE/Pool by a static greedy balancer using
    the cost model's per-engine rates (ACT 0.83 ns/elem + 185 ns/inst, DVE
    1.04 (0.52 with 2x) + 60-125, Pool 0.83 flat).
  - All input loads and output stores issue from the SP (sync) HWDGE queue,
    keeping ACT/DVE/Pool free for evacuation.
  - Software pipelining: pair p+1's load/normalize/transpose/Sv work is
    emitted in slices between pair p's four wedge column-groups so the PE
    queue never drains (PE p-state stays ramped) and the DVE norm work hides
    under the evacuation stream.
  - The PE is the critical path at ~61 us busy (256 wedge matmuls of 512
    cols + 16 Sv matmuls + 64 transposes); evacuation balances to ~53 us per
    engine, SP ~20 us.  Cost-model total ~65 us vs the 121.5 us baseline.
  - walrus encodes at most ONE semaphore wait on most instructions (and two
    on EventSemaphore), so `_spill_waits` post-processes the Tile-scheduled
    BIR, hoisting excess waits onto preceding same-engine EventSemaphores
    (sequencers run in order, so this is semantics-preserving).
"""

import numpy as np

B = 2
T = 2048
D = 1024
H = 16
Dh = 64
N_CORES = 8
PAIRS = (B * H) // N_CORES  # 4 per core
P = 128  # SBUF partitions

_COMPILED = {}

# test-harness knobs (default off; harness calls kernel() with these untouched)
TRACE = False
MM_DTYPE = "float32r"
LAST_RESULT = None


class _Balancer:
    """Static greedy assignment of PSUM-evacuation copies (and other movable
    elementwise work) to ACT/DVE/Pool using the v1 cost model's rates."""

    def __init__(self, nc):
        self.nc = nc
        self.busy = {"act": 0.0, "dve": 0.0, "pool": 0.0}

    def add_fixed(self, eng, ns):
        self.busy[eng] += ns

    @staticmethod
    def _cost(eng, free, psum_src, dve_2x):
        if eng == "act":
            return free * 0.8333 + 185.0
        if eng == "dve":
            mult = 0.5 if dve_2x else 1.0
            init = 125.0 if psum_src else 60.0
            return free * 1.0417 * mult + init
        return free * 0.8333  # pool

    def copy(self, out, in_, free, psum_src=True, dve_2x=False, engines=("act", "dve", "pool")):
        best = min(engines, key=lambda e: self.busy[e] + self._cost(e, free, psum_src, dve_2x))
        c = self._cost(best, free, psum_src, dve_2x)
        self.busy[best] += c
        nc = self.nc
        if best == "act":
            nc.scalar.copy(out, in_)
        elif best == "dve":
            nc.vector.tensor_copy(out, in_)
        else:
            nc.gpsimd.tensor_copy(out, in_)
        return best


def _build_nc(pairs=PAIRS, t=T, mm_dtype_name="float32r", spill=True, repeat=1):
    _import_concourse()
    from contextlib import ExitStack

    import concourse.bass as bass
    import concourse.tile as tile
    from concourse import mybir

    f32 = mybir.dt.float32
    bf16 = mybir.dt.bfloat16
    nt = t // P  # m-blocks per pair (16)
    ng = t // 512  # 512-wide col groups (4)
    W = 512

    nc = bass.Bass()
    x_in = nc.declare_dram_parameter("x", [pairs, t, Dh], f32, isOutput=False)
    s_in = nc.declare_dram_parameter("s", [pairs, Dh, Dh], f32, isOutput=False)
    id_in = nc.declare_dram_parameter("ident", [P, P], f32, isOutput=False)
    out_d = nc.declare_dram_parameter("out", [pairs, t, t], f32, isOutput=True)

    with ExitStack() as ctx:
        tc = ctx.enter_context(tile.TileContext(nc))
        const_pool = ctx.enter_context(tc.tile_pool(name="const", bufs=1))
        v_pool = ctx.enter_context(tc.tile_pool(name="v", bufs=2))
        n_pool = ctx.enter_context(tc.tile_pool(name="norm", bufs=2))
        vt_pool = ctx.enter_context(tc.tile_pool(name="vt", bufs=3))
        ob_pool = ctx.enter_context(tc.tile_pool(name="outb", bufs=4))
        psw_pool = ctx.enter_context(tc.tile_pool(name="psw", bufs=3, space="PSUM"))
        psa_pool = ctx.enter_context(tc.tile_pool(name="psa", bufs=2, space="PSUM"))

        bal = _Balancer(nc)
        consts = {}

        def emit_consts():
            # identity (bf16, for PE transposes), S -> bf16, warmups
            id_dma = const_pool.tile([P, P], f32)
            nc.scalar.dma_start(out=id_dma, in_=id_in[:, :])
            id16 = const_pool.tile([P, P], bf16)
            nc.vector.tensor_copy(id16, id_dma)
            bal.add_fixed("dve", 127.0)
            # all pairs' S as bf16 via a single gpsimd cast DMA
            s16 = const_pool.tile([Dh, pairs, Dh], bf16)
            nc.gpsimd.dma_start(
                out=s16, in_=s_in[:, :, :].rearrange("p d e -> d p e")
            )
            bal.add_fixed("pool", 500.0)
            # ACT table warm (sqrt_and_others holds both Sqrt and Copy)
            act_warm = const_pool.tile([1, 1], f32)
            nc.scalar.activation(
                act_warm, id_dma[:1, :1], mybir.ActivationFunctionType.Sqrt
            )
            bal.add_fixed("act", 1500.0)
            # PE p-state pre-warm: ~3us of dummy matmuls on one slot so the
            # first real transposes/matmuls run at the full 2.4 GHz p-state
            ps_warm = psw_pool.tile([P, 1024], f32, tag="psw", name="ps_warm")
            nc.tensor.matmul(
                ps_warm[:1, :1],
                lhsT=id16[:1, :1],
                rhs=id16[:1, :1],
                start=True,
                stop=True,
            )
            consts["id16"] = id16
            consts["s16"] = s16

        # ---------- per-pair phase A: load + normalize + transpose + Sv ----
        state = {}

        def emit_A_load(p, half, first=False):
            """half 0: chunks 0-1, half 1: chunks 2-3 (each chunk = 512 rows)."""
            if half == 0:
                state[p] = {
                    "v": v_pool.tile([P, nt, Dh], f32, tag="v", name="v_sb"),
                    "v16": n_pool.tile([P, nt, Dh], bf16, tag="v16", name="v16"),
                    "sq": n_pool.tile([P, nt, Dh], bf16, tag="sq", name="sq16"),
                    "ss": n_pool.tile([P, nt], f32, tag="ss", name="ss"),
                    "nrm": n_pool.tile([P, nt], f32, tag="nrm", name="nrm"),
                    "rinv": n_pool.tile([P, nt], f32, tag="rinv", name="rinv"),
                    "rinv16": n_pool.tile([P, nt], bf16, tag="rinv16", name="rinv16"),
                    "vh": n_pool.tile([P, nt, Dh], bf16, tag="vh", name="vh16"),
                    "vt": vt_pool.tile([Dh, t], bf16, tag="vt", name="vt16"),
                    "svt": vt_pool.tile([Dh, t], bf16, tag="svt", name="svt16"),
                }
            st = state[p]
            gn = nt // ng  # n-tiles per 512-row chunk (4)
            for g in (0, 1) if half == 0 else (2, 3):
                sl = slice(g * gn, (g + 1) * gn)
                # pair 0 is the pipeline fill: spread chunk loads over queues
                ld = (nc.sync, nc.scalar, nc.gpsimd, nc.sync)[g] if first else nc.sync
                if first and g == 0:
                    # halve the first chunk load so the norm chain starts early
                    for q in range(2):
                        sq_ = slice(2 * q, 2 * q + 2)
                        nc.sync.dma_start(
                            out=st["v"][:, sq_, :],
                            in_=x_in[p][q * 256 : (q + 1) * 256, :].rearrange(
                                "(n p) d -> p n d", p=P
                            ),
                        )
                else:
                    ld.dma_start(
                        out=st["v"][:, sl, :],
                        in_=x_in[p][g * 512 : (g + 1) * 512, :].rearrange(
                            "(n p) d -> p n d", p=P
                        ),
                    )
                # cast to bf16 (DVE 2x), square (all-bf16 TT, DVE 2x)
                nc.vector.tensor_copy(st["v16"][:, sl, :], st["v"][:, sl, :])
                nc.vector.tensor_mul(
                    st["sq"][:, sl, :], st["v16"][:, sl, :], st["v16"][:, sl, :]
                )
                bal.add_fixed("dve", 2 * (256 * 0.52 + 60))
                # free-axis row-sum is DVE-only
                nc.vector.reduce_sum(
                    st["ss"][:, sl], st["sq"][:, sl, :], axis=mybir.AxisListType.X
                )
                bal.add_fixed("dve", 256 * 1.0417 + 60)
                # per-chunk sqrt/recip/cast so transposes unblock early
                nc.scalar.activation(
                    st["nrm"][:, sl], st["ss"][:, sl], mybir.ActivationFunctionType.Sqrt
                )
                bal.add_fixed("act", gn * 0.8333 + 185)
                nc.vector.reciprocal(st["rinv"][:, sl], st["nrm"][:, sl])
                nc.vector.tensor_copy(st["rinv16"][:, sl], st["rinv"][:, sl])
                bal.add_fixed("dve", 130)

        def emit_A_tr(p, g):
            """normalize chunk g, transpose its 4 n-tiles, Sv matmul, evac."""
            st = state[p]
            gn = nt // ng
            sl = slice(g * gn, (g + 1) * gn)
            rb = st["rinv16"][:, sl].unsqueeze(-1).broadcast_to((P, gn, Dh))
            nc.vector.tensor_mul(st["vh"][:, sl, :], st["v16"][:, sl, :], rb)
            bal.add_fixed("dve", 256 * 0.52 + 60)
            ps = psa_pool.tile([P, W], f32, tag="psa", name="ps_a")
            ps_vt = ps.bitcast(bf16)[:Dh, :W]
            for j in range(gn):
                n = g * gn + j
                nc.tensor.transpose(
                    ps_vt[:, j * P : (j + 1) * P], st["vh"][:, n, :], consts["id16"]
                )
            bal.copy(st["vt"][:, g * W : (g + 1) * W], ps_vt, W, psum_src=True, dve_2x=True)
            ps_sv = ps[Dh:, :W]
            nc.tensor.matmul(
                ps_sv,
                lhsT=consts["s16"][:, p, :],
                rhs=st["vt"][:, g * W : (g + 1) * W],
                start=True,
                stop=True,
            )
            bal.copy(st["svt"][:, g * W : (g + 1) * W], ps_sv, W, psum_src=True)

        # ------ per-pair phase B: one (row-half, 512-col group) sub-block --
        # Row interleave within a half: t = h*1024 + q*8 + m, so the wedge
        # m-block of half h uses lhsT = svt[:, h*1024 + m : h*1024+1024 : 8]
        # (only that half of SvT -> half-barrier on phase A).
        nh = nt // 2  # m-chunks per half (8)

        def emit_B_block(p, h, g, last=False):
            st = state[p]
            ob = ob_pool.tile([P, nh, 516], f32, tag="ob", name="ob")
            rhs = st["vt"][:, g * W : (g + 1) * W]
            base = h * 1024
            for mm in range(0, nh, 2):
                ps_w = psw_pool.tile([P, 1024], f32, tag="psw", name="ps_w")
                for ms in range(2):
                    m = mm + ms
                    nc.tensor.matmul(
                        ps_w[:, ms * W : (ms + 1) * W],
                        lhsT=st["svt"][:, base + m : base + 1024 : nh],
                        rhs=rhs,
                        start=True,
                        stop=True,
                    )
                if last and mm == nh - 2:
                    # drain: split the final evacuation across all engines
                    nc.scalar.copy(ob[:, mm, :W], ps_w[:, :W])
                    nc.gpsimd.tensor_copy(ob[:, mm + 1, :256], ps_w[:, W : W + 256])
                    nc.vector.tensor_copy(ob[:, mm + 1, 256:W], ps_w[:, W + 256 :])
                else:
                    bal.copy(ob[:, mm : mm + 2, :W], ps_w, 1024, psum_src=True)
            # flat store: DRAM-side AP balances to [[rows,1024],[1,1],[1,W]]
            nc.sync.dma_start(
                out=out_d[p][base : base + 1024, g * W : (g + 1) * W],
                in_=ob[:, :, :W],
            )

        # ---------- emission with cross-pair software pipelining -----------
        plist = [q for _ in range(repeat) for q in range(pairs)]

        def emit_A_slice(p, i, first=False):
            if i == 0:
                emit_A_load(p, 0, first=first)
            elif i == 1:
                emit_A_load(p, 1, first=first)
            elif i == 2:
                emit_A_tr(p, 0)
                emit_A_tr(p, 1)
            else:
                emit_A_tr(p, 2)
                emit_A_tr(p, 3)

        emit_A_slice(plist[0], 0, first=True)
        emit_consts()
        emit_A_tr(plist[0], 0)
        emit_A_slice(plist[0], 1, first=True)
        emit_A_tr(plist[0], 1)
        emit_A_tr(plist[0], 2)
        emit_A_tr(plist[0], 3)
        for idx, p in enumerate(plist):
            nxt = plist[idx + 1] if idx + 1 < len(plist) else None
            for h in range(2):
                for g in range(ng):
                    emit_B_block(p, h, g, last=(nxt is None and h == 1 and g == ng - 1))
                    if nxt is not None and h == 0:
                        emit_A_slice(nxt, g)

    if spill:
        _spill_waits(nc)
    return nc


def _spill_waits(nc, multi_ok=("EventSemaphore",), max_keep=1):
    """Walrus encodes at most one sync-wait on Matmult (embedded weight load)
    and DMACopy; move extra waits onto a preceding same-engine EventSemaphore
    (which supports many waits). The engine sequencer processes instructions
    in order, so a preceding wait is semantically identical."""
    from concourse import mybir

    n_spilled = 0
    for f in nc.m.functions:
        for bb in f.blocks:
            il = bb.instructions
            out = []
            for inst in il:
                si = getattr(inst, "sync_info", None)
                waits = list((si.on_wait if si else None) or [])
                cap = 2 if inst.opcode in multi_ok else max_keep
                if len(waits) > cap:
                    moved, keep = waits[:-max_keep], waits[-max_keep:]
                    for k in range(0, len(moved), 2):
                        es = mybir.InstEventSemaphore(
                            name=f"{inst.name}-wspill{k}",
                            engine=inst.engine,
                            ins=[],
                            outs=[],
                            sync_info=mybir.SyncInfo(
                                on_wait=moved[k : k + 2], on_update=[]
                            ),
                        )
                        out.append(es)
                    inst.sync_info = mybir.SyncInfo(
                        on_wait=keep, on_update=list(si.on_update or [])
                    )
                    n_spilled += 1
                out.append(inst)
            il[:] = out
    return n_spilled


def _import_concourse():
    try:
        import concourse  # noqa: F401
    except ImportError:
        import sys

        for p in ("/opt/trn_rl_repo", "/root/.axon_site/_ro/trn_rl_repo"):
            if p not in sys.path:
                sys.path.insert(0, p)


def _ensure_device_backend():
    """If the process pinned JAX_PLATFORMS to cpu, lift the pin so the
    NeuronCores (axon platform) are reachable for the kernel run."""
    import os

    plats = os.environ.get("JAX_PLATFORMS", "")
    if plats and "axon" not in plats and "neuron" not in plats:
        os.environ["JAX_PLATFORMS"] = ""
        try:
            import jax

            jax.extend.backend.clear_backends()
        except Exception:
            pass


def kernel(x, A, window_size=None):
    _import_concourse()
    _ensure_device_backend()
    from concourse.bass_utils import run_bass_kernel_spmd

    x = np.ascontiguousarray(x, dtype=np.float32)
    A = np.ascontiguousarray(A, dtype=np.float32)
    assert x.shape == (B, T, D) and A.shape == (H, Dh, Dh)

    nc = _COMPILED.get(MM_DTYPE)
    if nc is None:
        nc = _build_nc(mm_dtype_name=MM_DTYPE)
        _COMPILED[MM_DTYPE] = nc

    # x[b, t, h*64:(h+1)*64] per (b,h) pair; pair index bh = b*H + h.
    xv = x.reshape(B, T, H, Dh).transpose(0, 2, 1, 3).reshape(B * H, T, Dh)
    S = (A - np.swapaxes(A, -1, -2)).astype(np.float32)  # replicated with heads
    S_all = np.tile(S, (B, 1, 1))
    ident = np.eye(P, dtype=np.float32)
    in_maps = []
    for c in range(N_CORES):
        sl = slice(c * PAIRS, (c + 1) * PAIRS)
        in_maps.append(
            {
                "x": np.ascontiguousarray(xv[sl]),
                "s": np.ascontiguousarray(S_all[sl]),
                "ident": ident,
            }
        )
    res = run_bass_kernel_spmd(nc, in_maps, list(range(N_CORES)), trace=TRACE)
    global LAST_RESULT
    LAST_RESULT = res
    outs = [res.results[c]["out"] for c in range(N_CORES)]
    full = np.concatenate(outs, axis=0).reshape(B, H, T, T)
    return full


# revision 14
# speedup vs baseline: 1.1091x; 1.1091x over previous
"""Trainium2 Bass kernel for nn_DirectionalWedgeBias.

Computes, per (batch b, head h):
    v      = x[b].reshape(T, H, Dh)[:, h, :]          # [T, Dh]
    v_hat  = v / max(||v||_2, eps)  (row-wise)
    S      = A[h] - A[h]^T                            # [Dh, Dh]
    wedge  = (v_hat @ S) @ v_hat^T                    # [T, T]

Full shapes: x [2, 2048, 1024] f32, A [16, 64, 64] f32 -> out [2, 16, 2048, 2048] f32.

Sharding: 32 independent (b, h) pairs split 4-per-core across 8 NeuronCores
(data + head parallel; the tiny skew-symmetric S is replicated/sliced with the
heads). Host pre-slices x into per-core [4, T, Dh] blocks, forms S = A - A^T,
and re-stacks the per-core [4, T, T] results.

Per-core dataflow (Tile framework), redesigned around the v1 cost model:

  - The wedge math runs in bf16 (v_hat, S, SvT all bf16; rel err ~5e-3 vs the
    2e-2 budget): bf16 matmuls and transposes run at 1 cyc/row on the PE, and
    DVE cast-copies/all-bf16 elementwise ops hit the 2x perf modes.
  - Interleaved m-blocks: wedge row-block m uses lhsT = svt16[:, m::16]
    (columns m, m+16, ...), so PSUM partition q of block m holds output row
    t = q*16 + m.  A group's 16 m-blocks staged at [128 part, 16, 512] in
    SBUF then flatten to *sequential* DRAM rows: the store's DRAM-side AP
    balances to [[rows, 2048], [1, 1], [1, 512]], whose free size (and hence
    DMA queue cost) is 512 elems -> ~0.8 us per 4 MiB store instead of the
    ~50 us a [128, ...]-leading AP costs.  The staging tile keeps a 516-elem
    chunk stride so its free dims cannot re-merge during AP balancing.
  - PSUM->SBUF evacuation (the 64 MiB wedge + SvT/vT) is the #2 cost center:
    copies are spread across ACT/DVE/Pool by a static greedy balancer using
    the cost model's per-engine rates (ACT 0.83 ns/elem + 185 ns/inst, DVE
    1.04 (0.52 with 2x) + 60-125, Pool 0.83 flat).
  - All input loads and output stores issue from the SP (sync) HWDGE queue,
    keeping ACT/DVE/Pool free for evacuation.
  - Software pipelining: pair p+1's load/normalize/transpose/Sv work is
    emitted in slices between pair p's four wedge column-groups so the PE
    queue never drains (PE p-state stays ramped) and the DVE norm work hides
    under the evacuation stream.
  - The PE is the critical path at ~61 us busy (256 wedge matmuls of 512
    cols + 16 Sv matmuls + 64 transposes); evacuation balances to ~53 us per
    engine, SP ~20 us.  Cost-model total ~65 us vs the 121.5 us baseline.
  - walrus encodes at most ONE semaphore wait on most instructions (and two
    on EventSemaphore), so `_spill_waits` post-processes the Tile-scheduled
    BIR, hoisting excess waits onto preceding same-engine EventSemaphores
    (sequencers run in order, so this is semantics-preserving).
"""

import numpy as np

B = 2
T = 2048
D = 1024
H = 16
Dh = 64
N_CORES = 8
PAIRS = (B * H) // N_CORES  # 4 per core
P = 128  # SBUF partitions

_COMPILED = {}

# test-harness knobs (default off; harness calls kernel() with these untouched)
TRACE = False
MM_DTYPE = "float32r"
LAST_RESULT = None


class _Balancer:
    """Static greedy assignment of PSUM-evacuation copies (and other movable
    elementwise work) to ACT/DVE/Pool using the v1 cost model's rates."""

    def __init__(self, nc):
        self.nc = nc
        self.busy = {"act": 0.0, "dve": 0.0, "pool": 0.0}

    def add_fixed(self, eng, ns):
        self.busy[eng] += ns

    @staticmethod
    def _cost(eng, free, psum_src, dve_2x):
        if eng == "act":
            return free * 0.8333 + 185.0
        if eng == "dve":
            mult = 0.5 if dve_2x else 1.0
            init = 125.0 if psum_src else 60.0
            return free * 1.0417 * mult + init
        return free * 0.8333  # pool

    def copy(self, out, in_, free, psum_src=True, dve_2x=False, engines=("act", "dve", "pool")):
        best = min(engines, key=lambda e: self.busy[e] + self._cost(e, free, psum_src, dve_2x))
        c = self._cost(best, free, psum_src, dve_2x)
        self.busy[best] += c
        nc = self.nc
        if best == "act":
            nc.scalar.copy(out, in_)
        elif best == "dve":
            nc.vector.tensor_copy(out, in_)
        else:
            nc.gpsimd.tensor_copy(out, in_)
        return best


def _build_nc(pairs=PAIRS, t=T, mm_dtype_name="float32r", spill=True, repeat=1):
    _import_concourse()
    from contextlib import ExitStack

    import concourse.bass as bass
    import concourse.tile as tile
    from concourse import mybir

    f32 = mybir.dt.float32
    bf16 = mybir.dt.bfloat16
    nt = t // P  # m-blocks per pair (16)
    ng = t // 512  # 512-wide col groups (4)
    W = 512

    nc = bass.Bass()
    x_in = nc.declare_dram_parameter("x", [pairs, t, Dh], f32, isOutput=False)
    s_in = nc.declare_dram_parameter("s", [pairs, Dh, Dh], f32, isOutput=False)
    id_in = nc.declare_dram_parameter("ident", [P, P], f32, isOutput=False)
    out_d = nc.declare_dram_parameter("out", [pairs, t, t], f32, isOutput=True)

    with ExitStack() as ctx:
        tc = ctx.enter_context(tile.TileContext(nc))
        const_pool = ctx.enter_context(tc.tile_pool(name="const", bufs=1))
        v_pool = ctx.enter_context(tc.tile_pool(name="v", bufs=2))
        n_pool = ctx.enter_context(tc.tile_pool(name="norm", bufs=2))
        vt_pool = ctx.enter_context(tc.tile_pool(name="vt", bufs=3))
        ob_pool = ctx.enter_context(tc.tile_pool(name="outb", bufs=4))
        psw_pool = ctx.enter_context(tc.tile_pool(name="psw", bufs=4, space="PSUM"))

        bal = _Balancer(nc)
        consts = {}

        def emit_consts():
            # identity (bf16, for PE transposes), S -> bf16, warmups
            id_dma = const_pool.tile([P, P], f32)
            nc.scalar.dma_start(out=id_dma, in_=id_in[:, :])
            id16 = const_pool.tile([P, P], bf16)
            nc.vector.tensor_copy(id16, id_dma)
            bal.add_fixed("dve", 127.0)
            # all pairs' S as bf16 via a single gpsimd cast DMA
            s16 = const_pool.tile([Dh, pairs, Dh], bf16)
            nc.gpsimd.dma_start(
                out=s16, in_=s_in[:, :, :].rearrange("p d e -> d p e")
            )
            bal.add_fixed("pool", 500.0)
            # ACT table warm (sqrt_and_others holds both Sqrt and Copy)
            act_warm = const_pool.tile([1, 1], f32)
            nc.scalar.activation(
                act_warm, id_dma[:1, :1], mybir.ActivationFunctionType.Sqrt
            )
            bal.add_fixed("act", 1500.0)
            # PE p-state pre-warm: ~3us of dummy matmuls on one slot so the
            # first real transposes/matmuls run at the full 2.4 GHz p-state
            ps_warm = psw_pool.tile([P, 1024], f32, tag="psw", name="ps_warm")
            nc.tensor.matmul(
                ps_warm[:1, :1],
                lhsT=id16[:1, :1],
                rhs=id16[:1, :1],
                start=True,
                stop=True,
            )
            consts["id16"] = id16
            consts["s16"] = s16

        # ---------- per-pair phase A: load + normalize + transpose + Sv ----
        state = {}

        def emit_A_load(p, half, first=False):
            """half 0: chunks 0-1, half 1: chunks 2-3 (each chunk = 512 rows)."""
            if half == 0:
                state[p] = {
                    "v": v_pool.tile([P, nt, Dh], f32, tag="v", name="v_sb"),
                    "v16": n_pool.tile([P, nt, Dh], bf16, tag="v16", name="v16"),
                    "sq": n_pool.tile([P, nt, Dh], bf16, tag="sq", name="sq16"),
                    "ss": n_pool.tile([P, nt], f32, tag="ss", name="ss"),
                    "nrm": n_pool.tile([P, nt], f32, tag="nrm", name="nrm"),
                    "rinv": n_pool.tile([P, nt], f32, tag="rinv", name="rinv"),
                    "rinv16": n_pool.tile([P, nt], bf16, tag="rinv16", name="rinv16"),
                    "vh": n_pool.tile([P, nt, Dh], bf16, tag="vh", name="vh16"),
                    "vt": vt_pool.tile([Dh, t], bf16, tag="vt", name="vt16"),
                    "svt": vt_pool.tile([Dh, t], bf16, tag="svt", name="svt16"),
                }
            st = state[p]
            gn = nt // ng  # n-tiles per 512-row chunk (4)
            for g in (0, 1) if half == 0 else (2, 3):
                sl = slice(g * gn, (g + 1) * gn)
                # pair 0 is the pipeline fill: spread chunk loads over queues
                ld = (nc.sync, nc.scalar, nc.gpsimd, nc.sync)[g] if first else nc.sync
                if first and g == 0:
                    # halve the first chunk load so the norm chain starts early
                    for q in range(2):
                        sq_ = slice(2 * q, 2 * q + 2)
                        nc.sync.dma_start(
                            out=st["v"][:, sq_, :],
                            in_=x_in[p][q * 256 : (q + 1) * 256, :].rearrange(
                                "(n p) d -> p n d", p=P
                            ),
                        )
                else:
                    ld.dma_start(
                        out=st["v"][:, sl, :],
                        in_=x_in[p][g * 512 : (g + 1) * 512, :].rearrange(
                            "(n p) d -> p n d", p=P
                        ),
                    )
                # cast to bf16 (DVE 2x), square (all-bf16 TT, DVE 2x)
                nc.vector.tensor_copy(st["v16"][:, sl, :], st["v"][:, sl, :])
                nc.vector.tensor_mul(
                    st["sq"][:, sl, :], st["v16"][:, sl, :], st["v16"][:, sl, :]
                )
                bal.add_fixed("dve", 2 * (256 * 0.52 + 60))
                # free-axis row-sum is DVE-only
                nc.vector.reduce_sum(
                    st["ss"][:, sl], st["sq"][:, sl, :], axis=mybir.AxisListType.X
                )
                bal.add_fixed("dve", 256 * 1.0417 + 60)
                # per-chunk sqrt/recip/cast so transposes unblock early
                nc.scalar.activation(
                    st["nrm"][:, sl], st["ss"][:, sl], mybir.ActivationFunctionType.Sqrt
                )
                bal.add_fixed("act", gn * 0.8333 + 185)
                nc.vector.reciprocal(st["rinv"][:, sl], st["nrm"][:, sl])
                nc.vector.tensor_copy(st["rinv16"][:, sl], st["rinv"][:, sl])
                bal.add_fixed("dve", 130)

        def emit_A_tr(p, g):
            """normalize chunk g, transpose its 4 n-tiles, Sv matmul, evac."""
            st = state[p]
            gn = nt // ng
            sl = slice(g * gn, (g + 1) * gn)
            rb = st["rinv16"][:, sl].unsqueeze(-1).broadcast_to((P, gn, Dh))
            nc.vector.tensor_mul(st["vh"][:, sl, :], st["v16"][:, sl, :], rb)
            bal.add_fixed("dve", 256 * 0.52 + 60)
            ps = psw_pool.tile([P, 1024], f32, tag="psw", name="ps_a")
            ps_vt = ps.bitcast(bf16)[:Dh, :W]
            for j in range(gn):
                n = g * gn + j
                nc.tensor.transpose(
                    ps_vt[:, j * P : (j + 1) * P], st["vh"][:, n, :], consts["id16"]
                )
            bal.copy(st["vt"][:, g * W : (g + 1) * W], ps_vt, W, psum_src=True, dve_2x=True)
            ps_sv = ps[Dh:, :W]
            nc.tensor.matmul(
                ps_sv,
                lhsT=consts["s16"][:, p, :],
                rhs=st["vt"][:, g * W : (g + 1) * W],
                start=True,
                stop=True,
            )
            bal.copy(st["svt"][:, g * W : (g + 1) * W], ps_sv, W, psum_src=True)

        # ------ per-pair phase B: one (row-half, 512-col group) sub-block --
        # Row interleave within a half: t = h*1024 + q*8 + m, so the wedge
        # m-block of half h uses lhsT = svt[:, h*1024 + m : h*1024+1024 : 8]
        # (only that half of SvT -> half-barrier on phase A).
        nh = nt // 2  # m-chunks per half (8)

        def emit_B_block(p, h, g, last=False):
            st = state[p]
            ob = ob_pool.tile([P, nh, 516], f32, tag="ob", name="ob")
            rhs = st["vt"][:, g * W : (g + 1) * W]
            base = h * 1024
            for mm in range(0, nh, 2):
                ps_w = psw_pool.tile([P, 1024], f32, tag="psw", name="ps_w")
                for ms in range(2):
                    m = mm + ms
                    nc.tensor.matmul(
                        ps_w[:, ms * W : (ms + 1) * W],
                        lhsT=st["svt"][:, base + m : base + 1024 : nh],
                        rhs=rhs,
                        start=True,
                        stop=True,
                    )
                if last and mm == nh - 2:
                    # drain: split the final evacuation across all engines
                    nc.scalar.copy(ob[:, mm, :W], ps_w[:, :W])
                    nc.gpsimd.tensor_copy(ob[:, mm + 1, :256], ps_w[:, W : W + 256])
                    nc.vector.tensor_copy(ob[:, mm + 1, 256:W], ps_w[:, W + 256 :])
                else:
                    bal.copy(ob[:, mm : mm + 2, :W], ps_w, 1024, psum_src=True)
            # flat store: DRAM-side AP balances to [[rows,1024],[1,1],[1,W]]
            nc.sync.dma_start(
                out=out_d[p][base : base + 1024, g * W : (g + 1) * W],
                in_=ob[:, :, :W],
            )

        # ---------- emission with cross-pair software pipelining -----------
        plist = [q for _ in range(repeat) for q in range(pairs)]

        def emit_A_slice(p, i, first=False):
            if i == 0:
                emit_A_load(p, 0, first=first)
            elif i == 1:
                emit_A_load(p, 1, first=first)
            elif i == 2:
                emit_A_tr(p, 0)
                emit_A_tr(p, 1)
            else:
                emit_A_tr(p, 2)
                emit_A_tr(p, 3)

        emit_A_slice(plist[0], 0, first=True)
        emit_consts()
        emit_A_tr(plist[0], 0)
        emit_A_slice(plist[0], 1, first=True)
        emit_A_tr(plist[0], 1)
        emit_A_tr(plist[0], 2)
        emit_A_tr(plist[0], 3)
        for idx, p in enumerate(plist):
            nxt = plist[idx + 1] if idx + 1 < len(plist) else None
            for h in range(2):
                for g in range(ng):
                    emit_B_block(p, h, g, last=(nxt is None and h == 1 and g == ng - 1))
                    if nxt is not None and h == 0:
                        emit_A_slice(nxt, g)

    if spill:
        _spill_waits(nc)
    return nc


def _spill_waits(nc, multi_ok=("EventSemaphore",), max_keep=1):
    """Walrus encodes at most one sync-wait on Matmult (embedded weight load)
    and DMACopy; move extra waits onto a preceding same-engine EventSemaphore
    (which supports many waits). The engine sequencer processes instructions
    in order, so a preceding wait is semantically identical."""
    from concourse import mybir

    n_spilled = 0
    for f in nc.m.functions:
        for bb in f.blocks:
            il = bb.instructions
            out = []
            for inst in il:
                si = getattr(inst, "sync_info", None)
                waits = list((si.on_wait if si else None) or [])
                cap = 2 if inst.opcode in multi_ok else max_keep
                if len(waits) > cap:
                    moved, keep = waits[:-max_keep], waits[-max_keep:]
                    for k in range(0, len(moved), 2):
                        es = mybir.InstEventSemaphore(
                            name=f"{inst.name}-wspill{k}",
                            engine=inst.engine,
                            ins=[],
                            outs=[],
                            sync_info=mybir.SyncInfo(
                                on_wait=moved[k : k + 2], on_update=[]
                            ),
                        )
                        out.append(es)
                    inst.sync_info = mybir.SyncInfo(
                        on_wait=keep, on_update=list(si.on_update or [])
                    )
                    n_spilled += 1
                out.append(inst)
            il[:] = out
    return n_spilled


def _import_concourse():
    try:
        import concourse  # noqa: F401
    except ImportError:
        import sys

        for p in ("/opt/trn_rl_repo", "/root/.axon_site/_ro/trn_rl_repo"):
            if p not in sys.path:
                sys.path.insert(0, p)


def _ensure_device_backend():
    """If the process pinned JAX_PLATFORMS to cpu, lift the pin so the
    NeuronCores (axon platform) are reachable for the kernel run."""
    import os

    plats = os.environ.get("JAX_PLATFORMS", "")
    if plats and "axon" not in plats and "neuron" not in plats:
        os.environ["JAX_PLATFORMS"] = ""
        try:
            import jax

            jax.extend.backend.clear_backends()
        except Exception:
            pass


def kernel(x, A, window_size=None):
    _import_concourse()
    _ensure_device_backend()
    from concourse.bass_utils import run_bass_kernel_spmd

    x = np.ascontiguousarray(x, dtype=np.float32)
    A = np.ascontiguousarray(A, dtype=np.float32)
    assert x.shape == (B, T, D) and A.shape == (H, Dh, Dh)

    nc = _COMPILED.get(MM_DTYPE)
    if nc is None:
        nc = _build_nc(mm_dtype_name=MM_DTYPE)
        _COMPILED[MM_DTYPE] = nc

    # x[b, t, h*64:(h+1)*64] per (b,h) pair; pair index bh = b*H + h.
    xv = x.reshape(B, T, H, Dh).transpose(0, 2, 1, 3).reshape(B * H, T, Dh)
    S = (A - np.swapaxes(A, -1, -2)).astype(np.float32)  # replicated with heads
    S_all = np.tile(S, (B, 1, 1))
    ident = np.eye(P, dtype=np.float32)
    in_maps = []
    for c in range(N_CORES):
        sl = slice(c * PAIRS, (c + 1) * PAIRS)
        in_maps.append(
            {
                "x": np.ascontiguousarray(xv[sl]),
                "s": np.ascontiguousarray(S_all[sl]),
                "ident": ident,
            }
        )
    res = run_bass_kernel_spmd(nc, in_maps, list(range(N_CORES)), trace=TRACE)
    global LAST_RESULT
    LAST_RESULT = res
    outs = [res.results[c]["out"] for c in range(N_CORES)]
    full = np.concatenate(outs, axis=0).reshape(B, H, T, T)
    return full


# revision 15
# speedup vs baseline: 1.1126x; 1.0032x over previous
"""Trainium2 Bass kernel for nn_DirectionalWedgeBias.

Computes, per (batch b, head h):
    v      = x[b].reshape(T, H, Dh)[:, h, :]          # [T, Dh]
    v_hat  = v / max(||v||_2, eps)  (row-wise)
    S      = A[h] - A[h]^T                            # [Dh, Dh]
    wedge  = (v_hat @ S) @ v_hat^T                    # [T, T]

Full shapes: x [2, 2048, 1024] f32, A [16, 64, 64] f32 -> out [2, 16, 2048, 2048] f32.

Sharding: 32 independent (b, h) pairs split 4-per-core across 8 NeuronCores
(data + head parallel; the tiny skew-symmetric S is replicated/sliced with the
heads). Host pre-slices x into per-core [4, T, Dh] blocks, forms S = A - A^T,
and re-stacks the per-core [4, T, T] results.

Per-core dataflow (Tile framework), redesigned around the v1 cost model:

  - The wedge math runs in bf16 (v_hat, S, SvT all bf16; rel err ~5e-3 vs the
    2e-2 budget): bf16 matmuls and transposes run at 1 cyc/row on the PE, and
    DVE cast-copies/all-bf16 elementwise ops hit the 2x perf modes.
  - Interleaved m-blocks: wedge row-block m uses lhsT = svt16[:, m::16]
    (columns m, m+16, ...), so PSUM partition q of block m holds output row
    t = q*16 + m.  A group's 16 m-blocks staged at [128 part, 16, 512] in
    SBUF then flatten to *sequential* DRAM rows: the store's DRAM-side AP
    balances to [[rows, 2048], [1, 1], [1, 512]], whose free size (and hence
    DMA queue cost) is 512 elems -> ~0.8 us per 4 MiB store instead of the
    ~50 us a [128, ...]-leading AP costs.  The staging tile keeps a 516-elem
    chunk stride so its free dims cannot re-merge during AP balancing.
  - PSUM->SBUF evacuation (the 64 MiB wedge + SvT/vT) is the #2 cost center:
    copies are spread across ACT/DVE/Pool by a static greedy balancer using
    the cost model's per-engine rates (ACT 0.83 ns/elem + 185 ns/inst, DVE
    1.04 (0.52 with 2x) + 60-125, Pool 0.83 flat).
  - All input loads and output stores issue from the SP (sync) HWDGE queue,
    keeping ACT/DVE/Pool free for evacuation.
  - Software pipelining: pair p+1's load/normalize/transpose/Sv work is
    emitted in slices between pair p's four wedge column-groups so the PE
    queue never drains (PE p-state stays ramped) and the DVE norm work hides
    under the evacuation stream.
  - The PE is the critical path at ~61 us busy (256 wedge matmuls of 512
    cols + 16 Sv matmuls + 64 transposes); evacuation balances to ~53 us per
    engine, SP ~20 us.  Cost-model total ~65 us vs the 121.5 us baseline.
  - walrus encodes at most ONE semaphore wait on most instructions (and two
    on EventSemaphore), so `_spill_waits` post-processes the Tile-scheduled
    BIR, hoisting excess waits onto preceding same-engine EventSemaphores
    (sequencers run in order, so this is semantics-preserving).
"""

import numpy as np

B = 2
T = 2048
D = 1024
H = 16
Dh = 64
N_CORES = 8
PAIRS = (B * H) // N_CORES  # 4 per core
P = 128  # SBUF partitions

_COMPILED = {}

# test-harness knobs (default off; harness calls kernel() with these untouched)
TRACE = False
MM_DTYPE = "float32r"
LAST_RESULT = None


class _Balancer:
    """Static greedy assignment of PSUM-evacuation copies (and other movable
    elementwise work) to ACT/DVE/Pool using the v1 cost model's rates."""

    def __init__(self, nc):
        self.nc = nc
        self.busy = {"act": 0.0, "dve": 0.0, "pool": 0.0}

    def add_fixed(self, eng, ns):
        self.busy[eng] += ns

    @staticmethod
    def _cost(eng, free, psum_src, dve_2x):
        if eng == "act":
            return free * 0.8333 + 185.0
        if eng == "dve":
            mult = 0.5 if dve_2x else 1.0
            init = 125.0 if psum_src else 60.0
            return free * 1.0417 * mult + init
        return free * 0.8333  # pool

    def copy(self, out, in_, free, psum_src=True, dve_2x=False, engines=("act", "dve", "pool")):
        best = min(engines, key=lambda e: self.busy[e] + self._cost(e, free, psum_src, dve_2x))
        c = self._cost(best, free, psum_src, dve_2x)
        self.busy[best] += c
        nc = self.nc
        if best == "act":
            nc.scalar.copy(out, in_)
        elif best == "dve":
            nc.vector.tensor_copy(out, in_)
        else:
            nc.gpsimd.tensor_copy(out, in_)
        return best


def _build_nc(pairs=PAIRS, t=T, mm_dtype_name="float32r", spill=True, repeat=1):
    _import_concourse()
    from contextlib import ExitStack

    import concourse.bass as bass
    import concourse.tile as tile
    from concourse import mybir

    f32 = mybir.dt.float32
    bf16 = mybir.dt.bfloat16
    nt = t // P  # m-blocks per pair (16)
    ng = t // 512  # 512-wide col groups (4)
    W = 512

    nc = bass.Bass()
    x_in = nc.declare_dram_parameter("x", [pairs, t, Dh], f32, isOutput=False)
    s_in = nc.declare_dram_parameter("s", [pairs, Dh, Dh], f32, isOutput=False)
    id_in = nc.declare_dram_parameter("ident", [P, P], f32, isOutput=False)
    out_d = nc.declare_dram_parameter("out", [pairs, t, t], f32, isOutput=True)

    with ExitStack() as ctx:
        tc = ctx.enter_context(tile.TileContext(nc))
        const_pool = ctx.enter_context(tc.tile_pool(name="const", bufs=1))
        v_pool = ctx.enter_context(tc.tile_pool(name="v", bufs=2))
        n_pool = ctx.enter_context(tc.tile_pool(name="norm", bufs=2))
        vt_pool = ctx.enter_context(tc.tile_pool(name="vt", bufs=3))
        ob_pool = ctx.enter_context(tc.tile_pool(name="outb", bufs=4))
        psw_pool = ctx.enter_context(tc.tile_pool(name="psw", bufs=4, space="PSUM"))

        bal = _Balancer(nc)
        consts = {}

        def emit_consts():
            # identity (bf16, for PE transposes), S -> bf16, warmups
            id_dma = const_pool.tile([P, P], f32)
            nc.scalar.dma_start(out=id_dma, in_=id_in[:, :])
            id16 = const_pool.tile([P, P], bf16)
            nc.vector.tensor_copy(id16, id_dma)
            bal.add_fixed("dve", 127.0)
            # all pairs' S as bf16 via a single gpsimd cast DMA
            s16 = const_pool.tile([Dh, pairs, Dh], bf16)
            nc.gpsimd.dma_start(
                out=s16, in_=s_in[:, :, :].rearrange("p d e -> d p e")
            )
            bal.add_fixed("pool", 500.0)
            # ACT table warm (sqrt_and_others holds both Sqrt and Copy)
            act_warm = const_pool.tile([1, 1], f32)
            nc.scalar.activation(
                act_warm, id_dma[:1, :1], mybir.ActivationFunctionType.Sqrt
            )
            bal.add_fixed("act", 1500.0)
            # PE p-state pre-warm: ~3us of dummy matmuls on one slot so the
            # first real transposes/matmuls run at the full 2.4 GHz p-state
            ps_warm = psw_pool.tile([P, 1024], f32, tag="psw", name="ps_warm")
            nc.tensor.matmul(
                ps_warm[:1, :1],
                lhsT=id16[:1, :1],
                rhs=id16[:1, :1],
                start=True,
                stop=True,
            )
            consts["id16"] = id16
            consts["s16"] = s16

        # ---------- per-pair phase A: load + normalize + transpose + Sv ----
        state = {}

        def emit_A_load(p, half, first=False):
            """half 0: chunks 0-1, half 1: chunks 2-3 (each chunk = 512 rows)."""
            if half == 0:
                state[p] = {
                    "v": v_pool.tile([P, nt, Dh], f32, tag="v", name="v_sb"),
                    "v16": n_pool.tile([P, nt, Dh], bf16, tag="v16", name="v16"),
                    "sq": n_pool.tile([P, nt, Dh], bf16, tag="sq", name="sq16"),
                    "ss": n_pool.tile([P, nt], f32, tag="ss", name="ss"),
                    "nrm": n_pool.tile([P, nt], f32, tag="nrm", name="nrm"),
                    "rinv": n_pool.tile([P, nt], f32, tag="rinv", name="rinv"),
                    "rinv16": n_pool.tile([P, nt], bf16, tag="rinv16", name="rinv16"),
                    "vh": n_pool.tile([P, nt, Dh], bf16, tag="vh", name="vh16"),
                    "vt": vt_pool.tile([Dh, t], bf16, tag="vt", name="vt16"),
                    "svt": vt_pool.tile([Dh, t], bf16, tag="svt", name="svt16"),
                }
            st = state[p]
            gn = nt // ng  # n-tiles per 512-row chunk (4)
            for g in (0, 1) if half == 0 else (2, 3):
                sl = slice(g * gn, (g + 1) * gn)
                # pair 0 is the pipeline fill: spread chunk loads over queues
                ld = (nc.sync, nc.scalar, nc.gpsimd, nc.sync)[g] if first else nc.sync
                if first and g == 0:
                    # halve the first chunk load so the norm chain starts early
                    for q in range(2):
                        sq_ = slice(2 * q, 2 * q + 2)
                        nc.sync.dma_start(
                            out=st["v"][:, sq_, :],
                            in_=x_in[p][q * 256 : (q + 1) * 256, :].rearrange(
                                "(n p) d -> p n d", p=P
                            ),
                        )
                else:
                    ld.dma_start(
                        out=st["v"][:, sl, :],
                        in_=x_in[p][g * 512 : (g + 1) * 512, :].rearrange(
                            "(n p) d -> p n d", p=P
                        ),
                    )
                # cast to bf16 (DVE 2x), square (all-bf16 TT, DVE 2x)
                nc.vector.tensor_copy(st["v16"][:, sl, :], st["v"][:, sl, :])
                nc.vector.tensor_mul(
                    st["sq"][:, sl, :], st["v16"][:, sl, :], st["v16"][:, sl, :]
                )
                bal.add_fixed("dve", 2 * (256 * 0.52 + 60))
                # free-axis row-sum is DVE-only
                nc.vector.reduce_sum(
                    st["ss"][:, sl], st["sq"][:, sl, :], axis=mybir.AxisListType.X
                )
                bal.add_fixed("dve", 256 * 1.0417 + 60)
                # per-chunk sqrt/recip/cast so transposes unblock early
                nc.scalar.activation(
                    st["nrm"][:, sl], st["ss"][:, sl], mybir.ActivationFunctionType.Sqrt
                )
                bal.add_fixed("act", gn * 0.8333 + 185)
                nc.vector.reciprocal(st["rinv"][:, sl], st["nrm"][:, sl])
                nc.vector.tensor_copy(st["rinv16"][:, sl], st["rinv"][:, sl])
                bal.add_fixed("dve", 130)

        def emit_A_tr(p, g):
            """normalize chunk g, transpose its 4 n-tiles, Sv matmul, evac."""
            st = state[p]
            gn = nt // ng
            sl = slice(g * gn, (g + 1) * gn)
            rb = st["rinv16"][:, sl].unsqueeze(-1).broadcast_to((P, gn, Dh))
            nc.vector.tensor_mul(st["vh"][:, sl, :], st["v16"][:, sl, :], rb)
            bal.add_fixed("dve", 256 * 0.52 + 60)
            ps = psw_pool.tile([P, 1024], f32, tag="psw", name="ps_a")
            ps_vt = ps.bitcast(bf16)[:Dh, :W]
            for j in range(gn):
                n = g * gn + j
                nc.tensor.transpose(
                    ps_vt[:, j * P : (j + 1) * P], st["vh"][:, n, :], consts["id16"]
                )
            bal.copy(st["vt"][:, g * W : (g + 1) * W], ps_vt, W, psum_src=True, dve_2x=True)
            ps_sv = ps[Dh:, :W]
            nc.tensor.matmul(
                ps_sv,
                lhsT=consts["s16"][:, p, :],
                rhs=st["vt"][:, g * W : (g + 1) * W],
                start=True,
                stop=True,
            )
            bal.copy(st["svt"][:, g * W : (g + 1) * W], ps_sv, W, psum_src=True)

        # ------ per-pair phase B: one (row-half, 512-col group) sub-block --
        # Row interleave within a half: t = h*1024 + q*8 + m, so the wedge
        # m-block of half h uses lhsT = svt[:, h*1024 + m : h*1024+1024 : 8]
        # (only that half of SvT -> half-barrier on phase A).
        nh = nt // 2  # m-chunks per half (8)

        def emit_B_block(p, h, g, last=False):
            st = state[p]
            ob = ob_pool.tile([P, nh, 516], f32, tag="ob", name="ob")
            rhs = st["vt"][:, g * W : (g + 1) * W]
            base = h * 1024
            for mm in range(0, nh, 2):
                ps_w = psw_pool.tile([P, 1024], f32, tag="psw", name="ps_w")
                for ms in range(2):
                    m = mm + ms
                    nc.tensor.matmul(
                        ps_w[:, ms * W : (ms + 1) * W],
                        lhsT=st["svt"][:, base + m : base + 1024 : nh],
                        rhs=rhs,
                        start=True,
                        stop=True,
                    )
                if last and mm == nh - 2:
                    # drain: split the final evacuation across all engines
                    nc.scalar.copy(ob[:, mm, :W], ps_w[:, :W])
                    nc.gpsimd.tensor_copy(ob[:, mm + 1, :256], ps_w[:, W : W + 256])
                    nc.vector.tensor_copy(ob[:, mm + 1, 256:W], ps_w[:, W + 256 :])
                else:
                    bal.copy(ob[:, mm : mm + 2, :W], ps_w, 1024, psum_src=True)
            # flat store: DRAM-side AP balances to [[rows,1024],[1,1],[1,W]]
            nc.sync.dma_start(
                out=out_d[p][base : base + 1024, g * W : (g + 1) * W],
                in_=ob[:, :, :W],
            )

        # ---------- emission with cross-pair software pipelining -----------
        plist = [q for _ in range(repeat) for q in range(pairs)]

        def emit_A_slice(p, i, first=False):
            if i == 0:
                emit_A_load(p, 0, first=first)
            elif i == 1:
                emit_A_load(p, 1, first=first)
            elif i == 2:
                emit_A_tr(p, 0)
                emit_A_tr(p, 1)
            else:
                emit_A_tr(p, 2)
                emit_A_tr(p, 3)

        emit_A_slice(plist[0], 0, first=True)
        emit_consts()
        emit_A_tr(plist[0], 0)
        emit_A_slice(plist[0], 1, first=True)
        emit_A_tr(plist[0], 1)
        emit_A_tr(plist[0], 2)
        emit_A_tr(plist[0], 3)
        for idx, p in enumerate(plist):
            nxt = plist[idx + 1] if idx + 1 < len(plist) else None
            for h in range(2):
                for g in range(ng):
                    if nxt is not None and h == 0:
                        emit_A_slice(nxt, g)
                    emit_B_block(p, h, g, last=(nxt is None and h == 1 and g == ng - 1))

    if spill:
        _spill_waits(nc)
    return nc


def _spill_waits(nc, multi_ok=("EventSemaphore",), max_keep=1):
    """Walrus encodes at most one sync-wait on Matmult (embedded weight load)
    and DMACopy; move extra waits onto a preceding same-engine EventSemaphore
    (which supports many waits). The engine sequencer processes instructions
    in order, so a preceding wait is semantically identical."""
    from concourse import mybir

    n_spilled = 0
    for f in nc.m.functions:
        for bb in f.blocks:
            il = bb.instructions
            out = []
            for inst in il:
                si = getattr(inst, "sync_info", None)
                waits = list((si.on_wait if si else None) or [])
                cap = 2 if inst.opcode in multi_ok else max_keep
                if len(waits) > cap:
                    moved, keep = waits[:-max_keep], waits[-max_keep:]
                    for k in range(0, len(moved), 2):
                        es = mybir.InstEventSemaphore(
                            name=f"{inst.name}-wspill{k}",
                            engine=inst.engine,
                            ins=[],
                            outs=[],
                            sync_info=mybir.SyncInfo(
                                on_wait=moved[k : k + 2], on_update=[]
                            ),
                        )
                        out.append(es)
                    inst.sync_info = mybir.SyncInfo(
                        on_wait=keep, on_update=list(si.on_update or [])
                    )
                    n_spilled += 1
                out.append(inst)
            il[:] = out
    return n_spilled


def _import_concourse():
    try:
        import concourse  # noqa: F401
    except ImportError:
        import sys

        for p in ("/opt/trn_rl_repo", "/root/.axon_site/_ro/trn_rl_repo"):
            if p not in sys.path:
                sys.path.insert(0, p)


def _ensure_device_backend():
    """If the process pinned JAX_PLATFORMS to cpu, lift the pin so the
    NeuronCores (axon platform) are reachable for the kernel run."""
    import os

    plats = os.environ.get("JAX_PLATFORMS", "")
    if plats and "axon" not in plats and "neuron" not in plats:
        os.environ["JAX_PLATFORMS"] = ""
        try:
            import jax

            jax.extend.backend.clear_backends()
        except Exception:
            pass


def kernel(x, A, window_size=None):
    _import_concourse()
    _ensure_device_backend()
    from concourse.bass_utils import run_bass_kernel_spmd

    x = np.ascontiguousarray(x, dtype=np.float32)
    A = np.ascontiguousarray(A, dtype=np.float32)
    assert x.shape == (B, T, D) and A.shape == (H, Dh, Dh)

    nc = _COMPILED.get(MM_DTYPE)
    if nc is None:
        nc = _build_nc(mm_dtype_name=MM_DTYPE)
        _COMPILED[MM_DTYPE] = nc

    # x[b, t, h*64:(h+1)*64] per (b,h) pair; pair index bh = b*H + h.
    xv = x.reshape(B, T, H, Dh).transpose(0, 2, 1, 3).reshape(B * H, T, Dh)
    S = (A - np.swapaxes(A, -1, -2)).astype(np.float32)  # replicated with heads
    S_all = np.tile(S, (B, 1, 1))
    ident = np.eye(P, dtype=np.float32)
    in_maps = []
    for c in range(N_CORES):
        sl = slice(c * PAIRS, (c + 1) * PAIRS)
        in_maps.append(
            {
                "x": np.ascontiguousarray(xv[sl]),
                "s": np.ascontiguousarray(S_all[sl]),
                "ident": ident,
            }
        )
    res = run_bass_kernel_spmd(nc, in_maps, list(range(N_CORES)), trace=TRACE)
    global LAST_RESULT
    LAST_RESULT = res
    outs = [res.results[c]["out"] for c in range(N_CORES)]
    full = np.concatenate(outs, axis=0).reshape(B, H, T, T)
    return full
